# revision 1
# baseline (speedup 1.0000x reference)
"""Trainium2 Bass kernel for nn_AttentionBlock (B=8, T=2048, C=512).

Data-parallel over batch: one batch element per NeuronCore (8 cores).

Per-core algorithm (batch b, x: [T, C]):
  qT = (Wq @ x^T + bq)      stored [D, T]   (feature-major)
  kT = (Wk @ x^T + bk)      stored [D, T]
  v  = (x @ Wv^T + bv)      stored [T, D]   (natural)
  stT[k, q] = sum_d kT[d,k] qT[d,q]         (scores transposed: [Tk, Tq])
  masked: valid iff q >= k  (causal); softmax over q = FREE axis of stT
  e[k, q] = exp((stT - max_q) / sqrt(D)); S[k] = sum_q e
  v_scaled[k, :] = v[k, :] / S[k]
  out[q, :] = sum_k e[k, q] * v_scaled[k, :]
  output = concat([x, out], axis=-1)        [T, 2C]

Matmul inputs are stored as float32r (fp32 data, reduced-precision PE pass
at full rate); producers write the rounded dtype directly so the BIR
verifier's fp32r rounding rule is satisfied. Causal structure is
exploited: score slices entirely in the masked region are never computed,
and the attn@v accumulation is triangular.
"""

import numpy as np

import concourse.bass as bass
import concourse.mybir as mybir
import concourse.tile as tile
from concourse import bacc

B, T, C = 8, 2048, 512
D = 512                      # KEY_SIZE == VALUE_SIZE == 512
P = 128                      # partitions
NT = T // P                  # 16 t-chunks
NC4 = C // P                 # 4 contraction chunks
ND = D // P                  # 4 d-chunks
QS = 512                     # q-slice width for score matmuls
NQ = T // QS                 # 4 q-slices
SCALE = float(1.0 / np.sqrt(D))
NEG = -1.0e30

F32 = mybir.dt.float32

# Scheduling knobs: every `copy_mod`-th PSUM->SBUF copy goes to ACT (0 = all
# DVE); engine for the v-row normalizer multiply.
CFG = {"copy_mod": 4, "vscale": "dve", "warmup": 0, "interleave_qproj": False,
       "acc_bufs": 4, "st_bufs": 4}

MM_DTYPES = {
    "f32r": mybir.dt.float32r,
    "f32": mybir.dt.float32,
    "bf16": mybir.dt.bfloat16,
}


def build_nc(mm_dtype="f32r"):
    """Build the single-core Bass program. mm_dtype in {"f32r", "f32", "bf16"}."""
    mdt = MM_DTYPES[mm_dtype]

    nc = bacc.Bacc(trn_type="TRN2", target_bir_lowering=False)

    x = nc.dram_tensor("x", [T, C], F32, kind="ExternalInput").ap()
    Wq = nc.dram_tensor("Wq", [D, C], F32, kind="ExternalInput").ap()
    bq = nc.dram_tensor("bq", [D], F32, kind="ExternalInput").ap()
    Wk = nc.dram_tensor("Wk", [D, C], F32, kind="ExternalInput").ap()
    bk = nc.dram_tensor("bk", [D], F32, kind="ExternalInput").ap()
    Wv = nc.dram_tensor("Wv", [D, C], F32, kind="ExternalInput").ap()
    bv = nc.dram_tensor("bv", [D], F32, kind="ExternalInput").ap()
    out = nc.dram_tensor("out", [T, 2 * C], F32, kind="ExternalOutput").ap()

    with tile.TileContext(nc) as tc:
        _emit(nc, tc, x, (Wq, bq), (Wk, bk), (Wv, bv), out, mdt)
    nc.compile()
    return nc


def _emit(nc, tc, x, wq, wk, wv, out, mdt):
    from contextlib import ExitStack

    Wq, bq = wq
    Wk, bk = wk
    Wv, bv = wv

    _ncopy = [0]

    def copy_ps(dst, src, eng=None, bias=None):
        """PSUM->SBUF copy on a chosen engine, with optional per-partition bias."""
        if eng is None:
            _ncopy[0] += 1
            mod = CFG["copy_mod"]
            eng = "act" if (mod and _ncopy[0] % mod == 0) else "dve"
        if eng == "dve":
            if bias is None:
                nc.vector.tensor_copy(dst, src)
            else:
                nc.vector.tensor_scalar_add(out=dst, in0=src, scalar1=bias)
        else:
            if bias is None:
                nc.scalar.activation(
                    out=dst, in_=src, func=mybir.ActivationFunctionType.Identity
                )
            else:
                nc.scalar.activation(
                    out=dst, in_=src,
                    func=mybir.ActivationFunctionType.Identity, bias=bias,
                )

    with ExitStack() as ctx:
        const = ctx.enter_context(tc.tile_pool(name="const", bufs=1))
        persist = ctx.enter_context(tc.tile_pool(name="persist", bufs=1))
        stats = ctx.enter_context(tc.tile_pool(name="stats", bufs=4))
        outsb = ctx.enter_context(tc.tile_pool(name="outsb", bufs=3))
        psum_acc = ctx.enter_context(
            tc.tile_pool(name="psum_acc", bufs=CFG["acc_bufs"], space="PSUM")
        )
        psum_st = ctx.enter_context(
            tc.tile_pool(name="psum_st", bufs=CFG["st_bufs"], space="PSUM")
        )

        # ---- constants ----
        ident = const.tile([P, P], F32, name="ident")
        nc.gpsimd.memset(ident, 0.0)
        nc.gpsimd.affine_select(
            out=ident, in_=ident, compare_op=mybir.AluOpType.not_equal,
            fill=1.0, base=0, pattern=[[-1, P]], channel_multiplier=1,
        )
        # tri[p, j] = 0 where j >= p (valid), NEG where j < p (masked)
        tri = const.tile([P, P], F32, name="tri")
        nc.gpsimd.memset(tri, 0.0)
        nc.gpsimd.affine_select(
            out=tri, in_=tri, compare_op=mybir.AluOpType.is_ge,
            fill=NEG, base=0, pattern=[[1, P]], channel_multiplier=-1,
        )
        # bv broadcast to all partitions: one rank-1 matmul (ones x bv) into
        # PSUM, copied once; v-projection copies then add it as tensor_add.
        ones_f = const.tile([1, P], F32, name="ones_f")
        nc.gpsimd.memset(ones_f, 1.0)
        bv_f = const.tile([1, D], F32, name="bv_f")
        nc.gpsimd.dma_start(out=bv_f, in_=bv.unsqueeze(0))
        bv_full = const.tile([P, D], F32, name="bv_full")
        ps_bv = psum_acc.tile([P, D], F32, name="ps_bv", tag="acc")
        nc.tensor.matmul(ps_bv, ones_f, bv_f, start=True, stop=True)
        nc.vector.tensor_copy(bv_full, ps_bv)

        bq_sb = const.tile([P, ND], F32, name="bq_sb")
        bk_sb = const.tile([P, ND], F32, name="bk_sb")
        for dc in range(ND):
            nc.gpsimd.dma_start(
                out=bq_sb[:, dc : dc + 1],
                in_=bq[dc * P : (dc + 1) * P].unsqueeze(-1),
            )
            nc.gpsimd.dma_start(
                out=bk_sb[:, dc : dc + 1],
                in_=bk[dc * P : (dc + 1) * P].unsqueeze(-1),
            )

        # ---- persistent activations (matmul dtype) ----
        qT = [persist.tile([P, T], mdt, name=f"qT{i}", tag=f"qT{i}") for i in range(ND)]
        kT = [persist.tile([P, T], mdt, name=f"kT{i}", tag=f"kT{i}") for i in range(ND)]
        v = [persist.tile([P, D], mdt, name=f"v{i}", tag=f"v{i}") for i in range(NT)]

        # ---- phase 0: load + transpose weights and x (plain fp32 PE transpose) ----
        with tc.tile_pool(name="wx", bufs=1) as wx, \
             tc.tile_pool(name="loads", bufs=1) as loads:
            wqT = [wx.tile([P, D], mdt, name=f"wqT{i}", tag=f"wqT{i}") for i in range(NC4)]
            wkT = [wx.tile([P, D], mdt, name=f"wkT{i}", tag=f"wkT{i}") for i in range(NC4)]
            wvT = [wx.tile([P, D], mdt, name=f"wvT{i}", tag=f"wvT{i}") for i in range(NC4)]
            xT = [wx.tile([P, T], mdt, name=f"xT{i}", tag=f"xT{i}") for i in range(NC4)]

            # During phases 0/1 the score pool is idle; alternate accumulation
            # groups across both PSUM pools so all 8 banks pipeline.
            psn = [0]

            def acc_tile(w=D, name="ps"):
                psn[0] += 1
                if psn[0] % 2:
                    return psum_acc.tile([P, w], F32, name=name, tag="acc")
                return psum_st.tile([P, w], F32, name=name, tag="st")

            if CFG.get("ident_prime", True):
                # First PE instruction: transpose ident into scratch PSUM.
                # It carries the Pool wait alone, so the following weight
                # transposes each carry a single DMA wait and are not
                # merged into one event-semaphore that waits on all loads.
                prime_ps = psum_st.tile([P, P], F32, name="prime_ps", tag="st")
                nc.tensor.transpose(prime_ps, ident, ident)

            def transpose_weight_bychunk(W, wT):
                # groups keyed by source chunk: each PSUM group depends on a
                # single DMA, so PE starts at the first chunk's arrival
                for dc in range(ND):
                    wn = loads.tile([P, C], F32, name=f"wnat{dc}",
                                    tag=f"wn{dc}", bufs=2)
                    nc.sync.dma_start(out=wn, in_=W[dc * P : (dc + 1) * P, :])
                    ps = acc_tile(name="ps_wt1")
                    for cc in range(NC4):
                        nc.tensor.transpose(
                            ps[:, cc * P : (cc + 1) * P],
                            wn[:, cc * P : (cc + 1) * P],
                            ident,
                        )
                    for cc in range(NC4):
                        copy_ps(wT[cc][:, dc * P : (dc + 1) * P],
                                ps[:, cc * P : (cc + 1) * P])

            def transpose_weight(W, wT, wtag):
                if CFG.get("w_one_dma"):
                    wall = loads.tile([P, ND, C], F32, name="wall",
                                      tag="wall", bufs=2)
                    nc.sync.dma_start(
                        out=wall,
                        in_=W.rearrange("(a p) c -> p a c", p=P),
                    )
                    wnat = [wall[:, dc, :] for dc in range(ND)]
                else:
                    wnat = []
                    for dc in range(ND):
                        wn = loads.tile([P, C], F32, name=f"wnat{dc}",
                                        tag=f"wn{dc}", bufs=2)
                        nc.sync.dma_start(out=wn, in_=W[dc * P : (dc + 1) * P, :])
                        wnat.append(wn)
                for cc in range(NC4):
                    ps = acc_tile(name="ps_wt")
                    for dc in range(ND):
                        nc.tensor.transpose(
                            ps[:, dc * P : (dc + 1) * P],
                            wnat[dc][:, cc * P : (cc + 1) * P],
                            ident,
                        )
                    copy_ps(wT[cc], ps)

            # PE warm-up: a few dependency-free zero matmuls issued while the
            # first DMA loads are in flight, so the HAM clock-gate reaches
            # full rate before real work starts.
            if CFG["warmup"]:
                zs = loads.tile([P, P], F32, name="zs", tag="zs")
                nc.gpsimd.memset(zs, 0.0)
                wu_ps = psum_acc.tile([P, P], F32, name="wu_ps", tag="acc")
                for _ in range(CFG["warmup"]):
                    nc.tensor.matmul(wu_ps, zs, zs, start=True, stop=True)

            def proj_group(wT, b_sb, dst, dc, qs, name):
                ps = acc_tile(QS, name=f"ps_{name}")
                for cc in range(NC4):
                    nc.tensor.matmul(
                        ps,
                        wT[cc][:, dc * P : (dc + 1) * P],
                        xT[cc][:, qs * QS : (qs + 1) * QS],
                        start=(cc == 0),
                        stop=(cc == NC4 - 1),
                    )
                copy_ps(
                    dst[dc][:, qs * QS : (qs + 1) * QS], ps,
                    bias=b_sb[:, dc : dc + 1],
                )

            # Load order: Wq first (smallest useful unit), then x (the long
            # pole: its transposes + all projections depend on it), then Wk/Wv.
            # q-projections interleave with x-transposes per 512-column group.
            if CFG.get("wq_bychunk", False):
                transpose_weight_bychunk(Wq, wqT)
            else:
                transpose_weight(Wq, wqT, "wq")

            def x_group(tg):
                xnat = []
                for j in range(4):
                    tch = tg * 4 + j
                    xn = loads.tile([P, C], F32, name=f"xnat{j}", tag=f"xn{j}", bufs=CFG.get("xn_bufs", 2))
                    nc.sync.dma_start(out=xn, in_=x[tch * P : (tch + 1) * P, :])
                    xnat.append(xn)
                for cc in range(NC4):
                    ps = acc_tile(name="ps_xt")
                    for j in range(4):
                        nc.tensor.transpose(
                            ps[:, j * P : (j + 1) * P],
                            xnat[j][:, cc * P : (cc + 1) * P],
                            ident,
                        )
                    copy_ps(xT[cc][:, tg * C : (tg + 1) * C], ps)

            if CFG.get("load_mix"):
                # weight loads slotted between x-chunk groups: PE transposes
                # weights while waiting for the next x group to arrive
                x_group(0)
                transpose_weight(Wk, wkT, "wk")
                x_group(1)
                transpose_weight(Wv, wvT, "wv")
                x_group(2)
                x_group(3)
            else:
                for tg in range(4):
                    x_group(tg)
                    if CFG["interleave_qproj"] and tg >= 1:
                        for dc in range(ND):
                            proj_group(wqT, bq_sb, qT, dc, tg - 1, "q")
            for qs in ([3] if CFG["interleave_qproj"] else range(NQ)):
                for dc in range(ND):
                    proj_group(wqT, bq_sb, qT, dc, qs, "q")

            if not CFG.get("load_mix"):
                transpose_weight(Wk, wkT, "wk")
                transpose_weight(Wv, wvT, "wv")

            # ---- phase 1: remaining projections ----
            for dc in range(ND):
                for qs in range(NQ):
                    proj_group(wkT, bk_sb, kT, dc, qs, "k")

            # v natural: v[tc] = sum_cc xT[cc][:,tc-block].T @ wvT[cc]  + bv
            for tch in range(NT):
                ps = acc_tile(name="ps_v")
                for cc in range(NC4):
                    nc.tensor.matmul(
                        ps,
                        xT[cc][:, tch * P : (tch + 1) * P],
                        wvT[cc],
                        start=(cc == 0),
                        stop=(cc == NC4 - 1),
                    )
                nc.vector.tensor_add(v[tch], ps, bv_full)

        # x passthrough: out[:, 0:C] = x (DRAM->DRAM), emitted after all input
        # loads on the same queue so it fills the queue-idle compute window.
        for g in range(8):
            r0 = g * (T // 8)
            nc.sync.dma_start(
                out=out[r0 : r0 + T // 8, 0:C], in_=x[r0 : r0 + T // 8, :]
            )

        # ---- phase 2: scores (transposed) + column-softmax ----
        with tc.tile_pool(name="epool", bufs=1) as epool:
            e = [
                epool.tile([P, T - kc * P], mdt, name=f"e{kc}", tag=f"e{kc}")
                for kc in range(NT)
            ]

            for kc in range(NT):
                k0 = kc * P
                j0 = k0 // QS
                # q-slices covering the valid region [k0, T)
                slices = [(k0, (j0 + 1) * QS - k0)]
                for j in range(j0 + 1, NQ):
                    slices.append((j * QS, QS))
                ns = len(slices)

                # No max-subtraction: logits are (q.k)/sqrt(512) with unit-ish
                # inputs, |logit| < ~8, so exp cannot overflow fp32.
                sums = stats.tile([P, NQ + 1], F32, name="sums", tag="sums")
                if CFG.get("tri_mode", "psum") == "post" and ns < NQ:
                    nc.vector.memset(sums[:, ns:NQ], 0.0)
                for idx, (q0, w) in enumerate(slices):
                    st = psum_st.tile([P, w], F32, name="st", tag="st")
                    for dc in range(ND):
                        nc.tensor.matmul(
                            st,
                            kT[dc][:, k0 : k0 + P],
                            qT[dc][:, q0 : q0 + w],
                            start=(dc == 0),
                            stop=(dc == ND - 1),
                        )
                    if idx == 0 and CFG.get("tri_mode", "psum") == "psum":
                        # diagonal block: mask strict lower triangle (q < k)
                        nc.vector.tensor_add(st[:, 0:P], st[:, 0:P], tri)
                    if idx == 0 and CFG.get("tri_mode", "psum") == "post":
                        # exp the diagonal block without accumulation, zero
                        # the invalid triangle on GpSimd, then sum on DVE —
                        # PE -> {ACT, Pool, DVE} instead of PE->DVE->ACT.
                        nc.scalar.activation(
                            out=e[kc][:, 0:P], in_=st[:, 0:P],
                            func=mybir.ActivationFunctionType.Exp,
                            bias=0.0, scale=SCALE,
                        )
                        nc.gpsimd.affine_select(
                            out=e[kc][:, 0:P], in_=e[kc][:, 0:P],
                            compare_op=mybir.AluOpType.is_ge,
                            fill=0.0, base=0, pattern=[[1, P]],
                            channel_multiplier=-1,
                        )
                        nc.vector.reduce_sum(
                            out=sums[:, 0:1], in_=e[kc][:, 0:P],
                            axis=mybir.AxisListType.X,
                        )
                        if w > P:
                            nc.scalar.activation(
                                out=e[kc][:, P:w], in_=st[:, P:w],
                                func=mybir.ActivationFunctionType.Exp,
                                bias=0.0, scale=SCALE,
                                accum_out=sums[:, NQ : NQ + 1],
                            )
                        else:
                            nc.vector.memset(sums[:, NQ : NQ + 1], 0.0)
                        continue
                    nc.scalar.activation(
                        out=e[kc][:, q0 - k0 : q0 - k0 + w],
                        in_=st,
                        func=mybir.ActivationFunctionType.Exp,
                        bias=0.0,
                        scale=SCALE,
                        accum_out=sums[:, idx : idx + 1],
                    )

                # The phase-3 matmuls for every qc >= kc wait on v[kc]'s scale;
                # run this chain ahead of queued copies on DVE/ACT.
                with tc.high_priority():
                    S = stats.tile([P, 1], F32, name="S", tag="S")
                    sum_w = (NQ + 1) if CFG.get("tri_mode", "psum") == "post" else ns
                    nc.vector.reduce_sum(
                        out=S, in_=sums[:, 0:sum_w], axis=mybir.AxisListType.X
                    )
                    rs = stats.tile([P, 1], F32, name="rs", tag="rs")
                    nc.vector.reciprocal(out=rs, in_=S)
                    # fold 1/S into v rows (normalizer is per-k == per-v-row)
                    if CFG["vscale"] == "act":
                        nc.scalar.mul(out=v[kc], in_=v[kc], mul=rs)
                    else:
                        nc.vector.tensor_scalar_mul(out=v[kc], in0=v[kc], scalar1=rs)

            # ---- phase 3: out[qc] = sum_{kc<=qc} e[kc][:, qc-block].T @ v[kc] ----
            for qc in range(NT):
                ps = psum_acc.tile([P, D], F32, name="ps_o", tag="acc")
                for kc in range(qc + 1):
                    off = (qc - kc) * P
                    nc.tensor.matmul(
                        ps,
                        e[kc][:, off : off + P],
                        v[kc],
                        start=(kc == 0),
                        stop=(kc == qc),
                    )
                osb = outsb.tile([P, D], F32, name="osb")
                copy_ps(osb, ps)
                (nc.scalar if CFG.get("out_dma") == "act" else nc.sync).dma_start(
                    out=out[qc * P : (qc + 1) * P, C : 2 * C], in_=osb
                )


_NC_CACHE = {}


def _get_nc(mm_dtype="f32r"):
    if mm_dtype not in _NC_CACHE:
        _NC_CACHE[mm_dtype] = build_nc(mm_dtype)
    return _NC_CACHE[mm_dtype]


def kernel(**inputs):
    from concourse.bass_utils import run_bass_kernel_spmd

    nc = _get_nc()
    x = np.asarray(inputs["x"], dtype=np.float32)
    shared = {
        name: np.ascontiguousarray(np.asarray(inputs[name], dtype=np.float32))
        for name in ("Wq", "bq", "Wk", "bk", "Wv", "bv")
    }
    in_maps = [
        {"x": np.ascontiguousarray(x[b]), **shared} for b in range(B)
    ]
    res = run_bass_kernel_spmd(nc, in_maps, core_ids=list(range(B)))
    return np.stack([res.results[b]["out"] for b in range(B)], axis=0)



# revision 2
# speedup vs baseline: 1.4090x; 1.4090x over previous
"""Trainium2 Bass kernel for nn_AttentionBlock (B=8, T=2048, C=512).

Data-parallel over batch: one batch element per NeuronCore (8 cores).

All matmuls run in fp8e4 (e4m3) with MatmulPerfMode.DoubleRow: the PE
virtualizes to 128x256, so a 512-deep contraction takes 2 instructions
instead of 4, at ~174 ns per instruction (HW-measured) vs ~323 ns for
fp32r -- ~3.7x less PE time for the same work.

Per-core algorithm (batch b, x: [T, C]):
  xT8[c, t]   = fp8(x^T)                   via bf16 PE transpose + cast
  qT8[d, t]   = fp8(Wq @ x^T + bq)         DoubleRow proj, bias at copy
  kT8[d, t]   = fp8(Wk @ x^T + bk)
  vbf[t, d]   = bf16(x @ Wv^T + bv)
  st[k, q]    = sum_d kT8[d,k] qT8[d,q]    scores transposed, DR pairs over d
  masked: valid iff q >= k (causal); softmax over q = FREE axis of st
  e8[k, q]    = fp8(exp(st/sqrt(D)))       ACT exp, fp32 row-sums S via accum
  v8[k, :]    = fp8(vbf[k, :] * 64/S[k])   normalizer folded into v; the *64
                                           keeps weights out of fp8 subnormals
  out[q, :]   = (1/64) sum_k e8[k, q] v8[k, :]   DR pairs over k-chunks
  output = concat([x, out], axis=-1)       [T, 2C]

e8 is stored at absolute q offsets [128, kc, q] so AV DoubleRow pairs
(2j, 2j+1) read aligned q-blocks; the 8 never-written pair-diagonal blocks
e8[:, 2j+1, block 2j] are zeroed once so they contribute nothing.
"""

import numpy as np

import concourse.bass as bass
import concourse.mybir as mybir
import concourse.tile as tile
from concourse import bacc

B, T, C = 8, 2048, 512
D = 512                      # KEY_SIZE == VALUE_SIZE == 512
P = 128                      # partitions
NT = T // P                  # 16 t-chunks
NC4 = C // P                 # 4 contraction chunks
ND = D // P                  # 4 d-chunks
QS = 512                     # q-slice width for score matmuls
NQ = T // QS                 # 4 q-slices
SCALE = float(1.0 / np.sqrt(D))
NEG = -1.0e30
ESC = 64.0                   # fp8 range shift for attention weights

F32 = mybir.dt.float32
BF16 = mybir.dt.bfloat16
FP8 = mybir.dt.float8e4
DR = mybir.MatmulPerfMode.DoubleRow

CFG = {"copy_mod": 2}

MM_DTYPES = {"f32r": mybir.dt.float32r}  # vestigial (hw_time compat)


def build_nc(mm_dtype="f32r"):
    nc = bacc.Bacc(trn_type="TRN2", target_bir_lowering=False)

    x = nc.dram_tensor("x", [T, C], F32, kind="ExternalInput").ap()
    Wq = nc.dram_tensor("Wq", [D, C], F32, kind="ExternalInput").ap()
    bq = nc.dram_tensor("bq", [D], F32, kind="ExternalInput").ap()
    Wk = nc.dram_tensor("Wk", [D, C], F32, kind="ExternalInput").ap()
    bk = nc.dram_tensor("bk", [D], F32, kind="ExternalInput").ap()
    Wv = nc.dram_tensor("Wv", [D, C], F32, kind="ExternalInput").ap()
    bv = nc.dram_tensor("bv", [D], F32, kind="ExternalInput").ap()
    out = nc.dram_tensor("out", [T, 2 * C], F32, kind="ExternalOutput").ap()

    with tile.TileContext(nc) as tc:
        _emit(nc, tc, x, (Wq, bq), (Wk, bk), (Wv, bv), out, None)
    nc.compile()
    return nc


def _emit(nc, tc, x, wq, wk, wv, out, _mdt):
    from contextlib import ExitStack

    Wq, bq = wq
    Wk, bk = wk
    Wv, bv = wv

    _ncopy = [0]

    def copy_ps(dst, src, bias=None, eng=None):
        """PSUM->SBUF copy alternating DVE/ACT, optional per-partition bias."""
        if eng is None:
            _ncopy[0] += 1
            eng = "act" if (_ncopy[0] % CFG["copy_mod"] == 0) else "dve"
        if eng == "dve":
            if bias is None:
                nc.vector.tensor_copy(dst, src)
            else:
                nc.vector.tensor_scalar_add(out=dst, in0=src, scalar1=bias)
        else:
            if bias is None:
                nc.scalar.activation(
                    out=dst, in_=src, func=mybir.ActivationFunctionType.Identity
                )
            else:
                nc.scalar.activation(
                    out=dst, in_=src,
                    func=mybir.ActivationFunctionType.Identity, bias=bias,
                )

    with ExitStack() as ctx:
        const = ctx.enter_context(tc.tile_pool(name="const", bufs=1))
        persist = ctx.enter_context(tc.tile_pool(name="persist", bufs=1))
        stats = ctx.enter_context(tc.tile_pool(name="stats", bufs=4))
        outsb = ctx.enter_context(tc.tile_pool(name="outsb", bufs=3))
        psum_acc = ctx.enter_context(
            tc.tile_pool(name="psum_acc", bufs=4, space="PSUM")
        )
        psum_st = ctx.enter_context(
            tc.tile_pool(name="psum_st", bufs=4, space="PSUM")
        )

        # ---- constants ----
        identf = const.tile([P, P], F32, name="identf")
        nc.gpsimd.memset(identf, 0.0)
        nc.gpsimd.affine_select(
            out=identf, in_=identf, compare_op=mybir.AluOpType.not_equal,
            fill=1.0, base=0, pattern=[[-1, P]], channel_multiplier=1,
        )
        identb = const.tile([P, P], BF16, name="identb")
        nc.gpsimd.tensor_copy(identb, identf)
        # tri[p, j] = 0 where j >= p (valid), NEG where j < p (masked)
        tri = const.tile([P, P], F32, name="tri")
        nc.gpsimd.memset(tri, 0.0)
        nc.gpsimd.affine_select(
            out=tri, in_=tri, compare_op=mybir.AluOpType.is_ge,
            fill=NEG, base=0, pattern=[[1, P]], channel_multiplier=-1,
        )
        # bv broadcast to all partitions via rank-1 fp32 matmul
        ones_f = const.tile([1, P], F32, name="ones_f")
        nc.gpsimd.memset(ones_f, 1.0)
        bv_f = const.tile([1, D], F32, name="bv_f")
        nc.gpsimd.dma_start(out=bv_f, in_=bv.unsqueeze(0))
        bv_full = const.tile([P, D], F32, name="bv_full")
        ps_bv = psum_acc.tile([P, D], F32, name="ps_bv", tag="acc")
        nc.tensor.matmul(ps_bv, ones_f, bv_f, start=True, stop=True)
        nc.vector.tensor_copy(bv_full, ps_bv)

        bq_sb = const.tile([P, ND], F32, name="bq_sb")
        bk_sb = const.tile([P, ND], F32, name="bk_sb")
        for dc in range(ND):
            nc.gpsimd.dma_start(
                out=bq_sb[:, dc : dc + 1],
                in_=bq[dc * P : (dc + 1) * P].unsqueeze(-1),
            )
            nc.gpsimd.dma_start(
                out=bk_sb[:, dc : dc + 1],
                in_=bk[dc * P : (dc + 1) * P].unsqueeze(-1),
            )

        # ---- persistent fp8/bf16 activations ----
        xT8 = persist.tile([P, NC4, T], FP8, name="xT8", tag="xT8")
        qT8 = persist.tile([P, ND, T], FP8, name="qT8", tag="qT8")
        kT8 = persist.tile([P, ND, T], FP8, name="kT8", tag="kT8")
        e8 = persist.tile([P, NT, T], FP8, name="e8", tag="e8")
        vbf = persist.tile([P, NT, D], BF16, name="vbf", tag="vbf")
        v8 = persist.tile([P, NT, D], FP8, name="v8", tag="v8")
        wqT8 = persist.tile([P, NC4, D], FP8, name="wqT8", tag="wqT8")
        wkT8 = persist.tile([P, NC4, D], FP8, name="wkT8", tag="wkT8")
        wvT8 = persist.tile([P, NC4, D], FP8, name="wvT8", tag="wvT8")

        # zero the 8 pair-diagonal blocks of e8 that AV reads but exp never
        # writes: e8[:, 2j+1, block 2j]
        for j in range(NT // 2):
            kc = 2 * j + 1
            nc.gpsimd.memset(e8[:, kc, 2 * j * P : (2 * j + 1) * P], 0.0)

        # ---- phase 0: load x + weights, transpose (bf16), cast to fp8 ----
        with tc.tile_pool(name="loads", bufs=1) as loads:

            def transpose_weight(W, wT8):
                # one DMA per d-chunk; transposes grouped per OUTPUT c-block
                wbf = []
                for dc in range(ND):
                    wn = loads.tile([P, C], F32, name=f"wnat{dc}",
                                    tag=f"wn{dc}", bufs=2)
                    nc.sync.dma_start(out=wn, in_=W[dc * P : (dc + 1) * P, :])
                    wb = loads.tile([P, C], BF16, name=f"wbf{dc}",
                                    tag=f"wb{dc}", bufs=2)
                    nc.gpsimd.tensor_copy(wb, wn)
                    wbf.append(wb)
                for cc in range(NC4):
                    ps = psum_acc.tile([P, D], BF16, name="ps_wt", tag="acc")
                    for dc in range(ND):
                        nc.tensor.transpose(
                            ps[:, dc * P : (dc + 1) * P],
                            wbf[dc][:, cc * P : (cc + 1) * P],
                            identb,
                        )
                    copy_ps(wT8[:, cc, :], ps)

            def x_group(tg):
                xbf = []
                for j in range(4):
                    tch = tg * 4 + j
                    xn = loads.tile([P, C], F32, name=f"xnat{j}",
                                    tag=f"xn{j}", bufs=2)
                    nc.sync.dma_start(out=xn, in_=x[tch * P : (tch + 1) * P, :])
                    xb = loads.tile([P, C], BF16, name=f"xbf{j}",
                                    tag=f"xb{j}", bufs=2)
                    nc.gpsimd.tensor_copy(xb, xn)
                    xbf.append(xb)
                for cc in range(NC4):
                    ps = psum_st.tile([P, D], BF16, name="ps_xt", tag="st")
                    for j in range(4):
                        nc.tensor.transpose(
                            ps[:, j * P : (j + 1) * P],
                            xbf[j][:, cc * P : (cc + 1) * P],
                            identb,
                        )
                    copy_ps(xT8[:, cc, tg * C : (tg + 1) * C], ps)

            transpose_weight(Wq, wqT8)
            for tg in range(4):
                x_group(tg)
            transpose_weight(Wk, wkT8)
            transpose_weight(Wv, wvT8)

            # ---- phase 1: projections (fp8 DoubleRow) ----
            def proj_qk(wT8, b_sb, dst, dc, qs):
                ps = psum_acc.tile([P, QS], F32, name="ps_qk", tag="acc")
                for s in range(2):
                    nc.tensor.matmul(
                        ps,
                        wT8[:, 2 * s : 2 * s + 2, dc * P : (dc + 1) * P],
                        xT8[:, 2 * s : 2 * s + 2, qs * QS : (qs + 1) * QS],
                        start=(s == 0),
                        stop=(s == 1),
                        perf_mode=DR,
                    )
                copy_ps(
                    dst[:, dc, qs * QS : (qs + 1) * QS], ps,
                    bias=b_sb[:, dc : dc + 1],
                )

            for dc in range(ND):
                for qs in range(NQ):
                    proj_qk(wqT8, bq_sb, qT8, dc, qs)
            for dc in range(ND):
                for qs in range(NQ):
                    proj_qk(wkT8, bk_sb, kT8, dc, qs)

            # v natural: v[tch] = x @ Wv^T + bv   (bf16 out)
            for tch in range(NT):
                ps = psum_acc.tile([P, D], F32, name="ps_v", tag="acc")
                for s in range(2):
                    nc.tensor.matmul(
                        ps,
                        xT8[:, 2 * s : 2 * s + 2, tch * P : (tch + 1) * P],
                        wvT8[:, 2 * s : 2 * s + 2, :],
                        start=(s == 0),
                        stop=(s == 1),
                        perf_mode=DR,
                    )
                nc.vector.tensor_add(vbf[:, tch, :], ps, bv_full)

        # x passthrough: out[:, 0:C] = x (DRAM->DRAM)
        for g in range(8):
            r0 = g * (T // 8)
            nc.sync.dma_start(
                out=out[r0 : r0 + T // 8, 0:C], in_=x[r0 : r0 + T // 8, :]
            )

        # ---- phase 2: scores (transposed) + column-softmax ----
        for kc in range(NT):
            k0 = kc * P
            j0 = k0 // QS
            slices = [(k0, (j0 + 1) * QS - k0)]
            for j in range(j0 + 1, NQ):
                slices.append((j * QS, QS))
            ns = len(slices)

            sums = stats.tile([P, NQ], F32, name="sums", tag="sums")
            for idx, (q0, w) in enumerate(slices):
                st = psum_st.tile([P, w], F32, name="st", tag="st")
                for s in range(2):
                    nc.tensor.matmul(
                        st,
                        kT8[:, 2 * s : 2 * s + 2, k0 : k0 + P],
                        qT8[:, 2 * s : 2 * s + 2, q0 : q0 + w],
                        start=(s == 0),
                        stop=(s == 1),
                        perf_mode=DR,
                    )
                if idx == 0:
                    # diagonal block: mask strict lower triangle (q < k)
                    nc.vector.tensor_add(st[:, 0:P], st[:, 0:P], tri)
                nc.scalar.activation(
                    out=e8[:, kc, q0 : q0 + w],
                    in_=st,
                    func=mybir.ActivationFunctionType.Exp,
                    bias=0.0,
                    scale=SCALE,
                    accum_out=sums[:, idx : idx + 1],
                )

            with tc.high_priority():
                S = stats.tile([P, 1], F32, name="S", tag="S")
                nc.vector.reduce_sum(
                    out=S, in_=sums[:, 0:ns], axis=mybir.AxisListType.X
                )
                rs = stats.tile([P, 1], F32, name="rs", tag="rs")
                nc.vector.reciprocal(out=rs, in_=S)
                rs64 = stats.tile([P, 1], F32, name="rs64", tag="rs64")
                nc.vector.tensor_scalar_mul(out=rs64, in0=rs, scalar1=ESC)
                # v8[kc] = vbf[kc] * (64/S) -- normalizer folded into v
                nc.scalar.activation(
                    out=v8[:, kc, :], in_=vbf[:, kc, :],
                    func=mybir.ActivationFunctionType.Identity, scale=rs64,
                )

        # ---- phase 3: out[qc] = (1/64) sum_j e8-pair(j, qc).T @ v8-pair(j) ----
        for qc in range(NT):
            ps = psum_acc.tile([P, D], F32, name="ps_o", tag="acc")
            npair = qc // 2 + 1
            for j in range(npair):
                nc.tensor.matmul(
                    ps,
                    e8[:, 2 * j : 2 * j + 2, qc * P : (qc + 1) * P],
                    v8[:, 2 * j : 2 * j + 2, :],
                    start=(j == 0),
                    stop=(j == npair - 1),
                    perf_mode=DR,
                )
            osb = outsb.tile([P, D], F32, name="osb")
            nc.vector.tensor_scalar_mul(out=osb, in0=ps, scalar1=1.0 / ESC)
            nc.sync.dma_start(
                out=out[qc * P : (qc + 1) * P, C : 2 * C], in_=osb
            )


_NC_CACHE = {}


def _get_nc(mm_dtype="f32r"):
    if mm_dtype not in _NC_CACHE:
        _NC_CACHE[mm_dtype] = build_nc(mm_dtype)
    return _NC_CACHE[mm_dtype]


def kernel(**inputs):
    from concourse.bass_utils import run_bass_kernel_spmd

    nc = _get_nc()
    x = np.asarray(inputs["x"], dtype=np.float32)
    shared = {
        name: np.ascontiguousarray(np.asarray(inputs[name], dtype=np.float32))
        for name in ("Wq", "bq", "Wk", "bk", "Wv", "bv")
    }
    in_maps = [
        {"x": np.ascontiguousarray(x[b]), **shared} for b in range(B)
    ]
    res = run_bass_kernel_spmd(nc, in_maps, core_ids=list(range(B)))
    return np.stack([res.results[b]["out"] for b in range(B)], axis=0)
